# revision 26
# baseline (speedup 1.0000x reference)
"""Bilinear decoder kernel for Trainium2 (8 NeuronCores).

score_e = sigmoid(z[row_e] @ W @ z[col_e])  for 200k edges, d=512.

Strategy (host->device transfer over axon is ~40MB/s, so uploads are
sharded and the tables are rebuilt on-device with AllGathers):
  - Upload per core (~1.5MB vs ~41MB for a replicated-f32 design):
    z shard [1280,512] bf16 (1/8 of nodes), W shard [64,512] bf16
    (1/8 of rows), edge indices [16, 2*1568] int16.
  - Device: AllGather W (tiny) -> full W in SBUF. Load z^T via
    dma_start_transpose, matmul ZW_c = z_c @ W for the local 1280-node
    shard (tensor engine, bf16). One combined AllGather of [z_c; zw_c]
    [2560,512] -> interleaved table [z_0; zw_0; z_1; zw_1; ...]
    [20480,512] bf16 in a Shared-addr-space scratchpad (one HBM copy for
    all 8 cores on the chip).
  - Gather ZW[row_e] and Z[col_e] rows from the shared table via
    dma_gather, per-edge dot via DVE mul (bf16) + ACT
    copy-with-accumulate, sigmoid on ACT, f16 scores out.
  - Edges sharded 25000/core; node ids are remapped on host to the
    interleaved table layout (col c -> (c//1250)*2560 + c%1250, row r
    adds +1280).

Host-side work is layout-only: bf16 casts, shard slicing, index
wrap/remap, output unshard.
"""

import sys

if "/opt/trn_rl_repo" not in sys.path:
    sys.path.insert(0, "/opt/trn_rl_repo")

from dataclasses import dataclass

import numpy as np


@dataclass(frozen=True)
class Cfg:
    n_cores: int = 8
    d: int = 512              # embedding dim
    n_nodes: int = 10000      # node table rows
    e_total: int = 200000     # total edges
    gchunk: int = 512         # edges per dma_gather (SDMA packet limit:
    #                           512 rows = 32 descriptors/engine works,
    #                           1024+ faults the exec unit with
    #                           single_packet=True)
    big_gather: bool = False  # one dma_gather per slab (single_packet=False)

    @property
    def kb(self):
        return self.d // 128  # 4

    @property
    def nsh(self):
        return self.n_nodes // self.n_cores  # 1250 nodes per core

    @property
    def nshp(self):
        return ((self.nsh + 127) // 128) * 128  # 1280 padded

    @property
    def nblocks(self):
        return self.nshp // 128  # 10

    @property
    def ntab(self):
        return self.nshp * self.n_cores  # 10240 table rows

    @property
    def wsh(self):
        return self.d // self.n_cores  # 64 W rows per core

    @property
    def e_core(self):
        return self.e_total // self.n_cores  # 25000

    @property
    def ep_core(self):
        return ((self.e_core + 127) // 128) * 128  # 25088

    @property
    def eblocks(self):
        return self.ep_core // 128  # 196

    @property
    def idx_cols(self):
        return self.ep_core // 16  # 1568

    @property
    def chunks(self):
        out = []
        left = self.ep_core
        while left > 0:
            c = min(self.gchunk, left)
            out.append(c)
            left -= c
        return out

    @property
    def slab_blocks(self):
        # 196 edge-blocks in 6 slabs; slab tiles are [128, 33, 512] bf16
        # (33KB/partition, x2 tables x2 bufs = 132KB, fits SBUF beside the
        # other tiles and double-buffers gathers against the fused dots)
        return (self.eblocks + 5) // 6  # 33

    @property
    def slabs(self):
        out = []
        b = 0
        while b < self.eblocks:
            e = min(b + self.slab_blocks, self.eblocks)
            out.append((b, e))
            b = e
        return out


CFG = Cfg()


def build_kernel(cfg: Cfg):
    """Build + compile the Bacc module. Returns nc."""
    import concourse.bacc as bacc
    import concourse.mybir as mybir
    from concourse import tile

    f32 = mybir.dt.float32
    f16 = mybir.dt.float16
    bf16 = mybir.dt.bfloat16
    i16 = mybir.dt.int16

    D, KB, NSHP, NB = cfg.d, cfg.kb, cfg.nshp, cfg.nblocks
    NTAB, WSH, IC = cfg.ntab, cfg.wsh, cfg.idx_cols
    group = [list(range(cfg.n_cores))]

    nc = bacc.Bacc(
        "TRN2", target_bir_lowering=False, debug=False, num_devices=cfg.n_cores
    )

    zin = nc.dram_tensor("zin", [NSHP, D], bf16, kind="ExternalInput")
    win = nc.dram_tensor("win", [WSH, D], bf16, kind="ExternalInput")
    eidx = nc.dram_tensor("eidx", [16, 2 * IC], i16, kind="ExternalInput")
    scores = nc.dram_tensor("scores", [128, cfg.eblocks], f16, kind="ExternalOutput")
    # Shared-scratchpad AllGather outputs: one copy in chip HBM instead of
    # eight core-local replicas (supported for AllGather with 8 cores).
    wag_out = nc.dram_tensor("wag_out", [D, D], bf16, addr_space="Shared")
    zag_out = nc.dram_tensor("zag_out", [2 * NTAB, D], bf16, addr_space="Shared")

    with tile.TileContext(nc) as tc:
        with (
            tc.tile_pool(name="const", bufs=1) as constp,
            tc.tile_pool(name="dram", bufs=1, space="DRAM") as dramp,
            tc.tile_pool(name="zwsb", bufs=2) as zwp,
            tc.tile_pool(name="rows", bufs=2) as rowsp,
            tc.tile_pool(name="cols", bufs=2) as colsp,
            tc.tile_pool(name="ps", bufs=2, space="PSUM") as psp,
        ):
            # ---- DRAM bounce buffers (collectives can't touch I/O tensors) ----
            # Combined z+zw AllGather: each core contributes [z_c; zw_c]
            # [2*NSHP, D]; output is the interleaved table
            # [z_0; zw_0; z_1; zw_1; ...] that host-side index remap targets.
            wag_in = dramp.tile([WSH, D], bf16, tag="wag_in")
            zag_in = dramp.tile([2 * NSHP, D], bf16, tag="zag_in")

            nc.sync.dma_start(wag_in[:], win.ap())
            nc.sync.dma_start(zag_in[:NSHP, :], zin.ap())

            # ---- collectives (gpsimd, straight-line order) ----
            nc.gpsimd.collective_compute(
                "AllGather",
                mybir.AluOpType.bypass,
                replica_groups=group,
                ins=[wag_in.opt()],
                outs=[wag_out.ap()],
            )

            # ---- SBUF constants ----
            w_sb = constp.tile([128, KB, D], bf16, tag="w")
            nc.sync.dma_start(
                w_sb[:], wag_out.ap().rearrange("(kb p) f -> p kb f", p=128)
            )
            # z^T for the matmul's stationary operand: [128, kb, NSHP]
            zt_sb = constp.tile([128, KB, NSHP], bf16, tag="zt")
            nc.sync.dma_start_transpose(zt_sb[:], zin.ap())
            # edge indices: upload 16-partition wrap, replicate to 128
            idx_sb = constp.tile([128, 2 * IC], i16, tag="idx")
            nc.sync.dma_start(idx_sb[0:16, :], eidx.ap())
            for p in (16, 32, 64):  # replicate 16 -> 128 partitions by doubling
                nc.sync.dma_start(idx_sb[p : 2 * p, :], idx_sb[0:p, :])
            scores_sb = constp.tile([128, cfg.eblocks], f32, tag="scores")
            sig_sb = constp.tile([128, cfg.eblocks], f16, tag="sig")

            # ---- phase 1: ZW_c = z_c @ W for the local node shard ----
            # node blocks in pairs: 8 matmuls share one PSUM->SBUF copy and
            # one DMA (per-instruction dispatch overhead dominates on HW)
            for nb2 in range(NB // 2):
                ps = psp.tile([128, 2, D], f32, tag="ps")
                for h in range(2):
                    for kb in range(KB):
                        nc.tensor.matmul(
                            ps[:, h, :],
                            lhsT=zt_sb[:, kb, (2 * nb2 + h) * 128 : (2 * nb2 + h + 1) * 128],
                            rhs=w_sb[:, kb, :],
                            start=(kb == 0),
                            stop=(kb == KB - 1),
                        )
                zw_t = zwp.tile([128, 2, D], bf16, tag="zwt")
                nc.vector.tensor_copy(zw_t[:], ps[:])
                nc.sync.dma_start(
                    zag_in[NSHP + nb2 * 256 : NSHP + (nb2 + 1) * 256, :].rearrange(
                        "(b p) f -> p b f", p=128
                    ),
                    zw_t[:],
                )

            nc.gpsimd.collective_compute(
                "AllGather",
                mybir.AluOpType.bypass,
                replica_groups=group,
                ins=[zag_in.opt()],
                outs=[zag_out.ap()],
            )

            # ---- phase 2: gathers + per-edge dots ----
            # Real device time here is dominated by per-instruction dispatch
            # overhead, so the dot products are fused: all 196 edge-blocks are
            # processed in 3 slabs, each one giant in-place DVE multiply over
            # [128, nb, 512] plus one tensor_reduce -> [128, nb] score columns
            # (6 instructions instead of 392).
            for t, (b0, b1) in enumerate(cfg.slabs):
                nb = b1 - b0
                ctile = colsp.tile([128, cfg.slab_blocks, D], bf16, tag="ct")
                rtile = rowsp.tile([128, cfg.slab_blocks, D], bf16, tag="rt")
                gstep = nb * 128 if cfg.big_gather else cfg.gchunk
                for c0 in range(0, nb * 128, gstep):
                    G = min(gstep, nb * 128 - c0)
                    off = (b0 * 128 + c0) // 16
                    ob = c0 // 128
                    gb = G // 128
                    nc.gpsimd.dma_gather(
                        ctile[:, ob : ob + gb, :],
                        zag_out.ap(),
                        idx_sb[:, IC + off : IC + off + G // 16],
                        num_idxs=G,
                        num_idxs_reg=G,
                        elem_size=D,
                        single_packet=not cfg.big_gather,
                    )
                    nc.gpsimd.dma_gather(
                        rtile[:, ob : ob + gb, :],
                        zag_out.ap(),
                        idx_sb[:, off : off + G // 16],
                        num_idxs=G,
                        num_idxs_reg=G,
                        elem_size=D,
                        single_packet=not cfg.big_gather,
                    )
                nc.vector.tensor_mul(
                    rtile[:, :nb, :], rtile[:, :nb, :], ctile[:, :nb, :]
                )
                nc.vector.tensor_reduce(
                    scores_sb[:, b0:b1],
                    rtile[:, :nb, :],
                    axis=mybir.AxisListType.X,
                    op=mybir.AluOpType.add,
                )

            # ---- sigmoid + writeback ----
            nc.scalar.activation(
                sig_sb[:], scores_sb[:], mybir.ActivationFunctionType.Sigmoid
            )
            nc.sync.dma_start(scores.ap(), sig_sb[:])

    nc.compile()
    return nc


def _wrap_idx(ids: np.ndarray, cfg: Cfg) -> np.ndarray:
    """int table-row ids [ep_core] -> [16, ep_core//16] int16 in the
    16-partition wrapped layout dma_gather expects."""
    out = np.empty((16, cfg.ep_core // 16), dtype=np.int16)
    off = 0
    for G in cfg.chunks:
        c = ids[off : off + G].reshape(G // 16, 16).T  # [16, G/16]
        out[:, off // 16 : (off + G) // 16] = c
        off += G
    return out


def prep_inputs(z_drug, weight, batch_edges, cfg: Cfg):
    """Host-side layout prep. Returns per-core input maps."""
    import ml_dtypes

    bf16 = ml_dtypes.bfloat16

    z = np.asarray(z_drug, dtype=np.float32)
    w = np.asarray(weight, dtype=np.float32)
    be = np.asarray(batch_edges)

    per_core = []
    for c in range(cfg.n_cores):
        # z shard: nodes [c*1250, (c+1)*1250), padded to 1280 rows
        zsh = np.zeros((cfg.nshp, cfg.d), dtype=bf16)
        zsh[: cfg.nsh] = z[c * cfg.nsh : (c + 1) * cfg.nsh].astype(bf16)
        # W shard: rows [c*64, (c+1)*64)
        wsh = np.ascontiguousarray(
            w[c * cfg.wsh : (c + 1) * cfg.wsh].astype(bf16)
        )
        # edge shard + remap node ids to the padded AllGather table layout
        sl = slice(c * cfg.e_core, (c + 1) * cfg.e_core)
        rids = np.zeros(cfg.ep_core, dtype=np.int64)
        cids = np.zeros(cfg.ep_core, dtype=np.int64)
        rids[: cfg.e_core] = be[0, sl]
        cids[: cfg.e_core] = be[1, sl]
        # combined table layout: [z_0; zw_0; z_1; zw_1; ...], stride 2*nshp
        rids = (rids // cfg.nsh) * (2 * cfg.nshp) + cfg.nshp + rids % cfg.nsh
        cids = (cids // cfg.nsh) * (2 * cfg.nshp) + cids % cfg.nsh
        eidx = np.concatenate(
            [_wrap_idx(rids, cfg), _wrap_idx(cids, cfg)], axis=1
        )
        per_core.append({"zin": zsh, "win": wsh, "eidx": eidx})
    return per_core


_NC_CACHE = {}


def get_nc(cfg: Cfg):
    key = (cfg.gchunk,)
    if key not in _NC_CACHE:
        _NC_CACHE[key] = build_kernel(cfg)
    return _NC_CACHE[key]


def _unshard(results, cfg: Cfg) -> np.ndarray:
    parts = []
    for c in range(cfg.n_cores):
        raw = results[c]["scores"]  # [128, eblocks], edge i at [i%128, i//128]
        parts.append(raw.T.reshape(-1)[: cfg.e_core])
    return np.concatenate(parts).astype(np.float32)


def run(z_drug, weight, batch_edges, cfg: Cfg, repeats: int = 1):
    """Returns (scores[200000] f32, [wall seconds per call])."""
    import time

    from concourse.bass_utils import run_bass_kernel_spmd

    nc = get_nc(cfg)
    in_maps = prep_inputs(z_drug, weight, batch_edges, cfg)
    walls = []
    res = None
    for _ in range(max(1, repeats)):
        t0 = time.perf_counter()
        try:
            res = run_bass_kernel_spmd(
                nc, in_maps, core_ids=list(range(cfg.n_cores))
            )
        except Exception:
            if res is not None:
                break  # keep earlier good result; a repeat run hiccupped
            time.sleep(30)
            res = run_bass_kernel_spmd(
                nc, in_maps, core_ids=list(range(cfg.n_cores))
            )
        walls.append(time.perf_counter() - t0)
    return _unshard(res.results, cfg), walls


def kernel(z_drug, weight, batch_edges):
    out, _ = run(z_drug, weight, batch_edges, CFG)
    return out


# revision 29
# speedup vs baseline: 1.6779x; 1.6779x over previous
"""Bilinear decoder kernel for Trainium2 (8 NeuronCores).

score_e = sigmoid(z[row_e] @ W @ z[col_e])  for 200k edges, d=512.

Strategy (host->device transfer over axon is ~40MB/s, so uploads are
sharded and the tables are rebuilt on-device with AllGathers):
  - Upload per core (~1.5MB vs ~41MB for a replicated-f32 design):
    z shard [1280,512] bf16 (1/8 of nodes), W shard [64,512] bf16
    (1/8 of rows), edge indices [16, 2*1568] int16.
  - Device: AllGather W (tiny) -> full W in SBUF. Load z^T via
    dma_start_transpose, matmul ZW_c = z_c @ W for the local 1280-node
    shard (tensor engine, bf16). One combined AllGather of [z_c; zw_c]
    [2560,512] -> interleaved table [z_0; zw_0; z_1; zw_1; ...]
    [20480,512] bf16 in a Shared-addr-space scratchpad (one HBM copy for
    all 8 cores on the chip).
  - Gather ZW[row_e] and Z[col_e] rows from the shared table via
    dma_gather (512-row chunks) into 6 slabs of 33 edge-blocks; per-edge
    dots are FUSED per slab (one in-place DVE tensor_mul + one
    tensor_reduce over [128,33,512]) because real HW pays ~100us
    dispatch overhead per instruction; sigmoid on ACT, f16 scores out.
  - Edges sharded 25000/core; node ids are remapped on host to the
    interleaved table layout (col c -> (c//1250)*2560 + c%1250, row r
    adds +1280).

Host-side work is layout-only: bf16 casts, shard slicing, index
wrap/remap, output unshard.
"""

import sys

if "/opt/trn_rl_repo" not in sys.path:
    sys.path.insert(0, "/opt/trn_rl_repo")

from dataclasses import dataclass

import numpy as np


@dataclass(frozen=True)
class Cfg:
    n_cores: int = 8
    d: int = 512              # embedding dim
    n_nodes: int = 10000      # node table rows
    e_total: int = 200000     # total edges
    gchunk: int = 512         # edges per dma_gather (SDMA packet limit:
    #                           512 rows = 32 descriptors/engine works,
    #                           1024+ faults the exec unit with
    #                           single_packet=True)
    big_gather: bool = False  # one dma_gather per slab (single_packet=False)

    @property
    def kb(self):
        return self.d // 128  # 4

    @property
    def nsh(self):
        return self.n_nodes // self.n_cores  # 1250 nodes per core

    @property
    def nshp(self):
        return ((self.nsh + 127) // 128) * 128  # 1280 padded

    @property
    def nblocks(self):
        return self.nshp // 128  # 10

    @property
    def ntab(self):
        return self.nshp * self.n_cores  # 10240 table rows

    @property
    def wsh(self):
        return self.d // self.n_cores  # 64 W rows per core

    @property
    def e_core(self):
        return self.e_total // self.n_cores  # 25000

    @property
    def ep_core(self):
        return ((self.e_core + 127) // 128) * 128  # 25088

    @property
    def eblocks(self):
        return self.ep_core // 128  # 196

    @property
    def idx_cols(self):
        return self.ep_core // 16  # 1568

    @property
    def chunks(self):
        out = []
        left = self.ep_core
        while left > 0:
            c = min(self.gchunk, left)
            out.append(c)
            left -= c
        return out

    @property
    def slab_blocks(self):
        # 196 edge-blocks in 6 slabs; slab tiles are [128, 33, 512] bf16
        # (33KB/partition, x2 tables x2 bufs = 132KB, fits SBUF beside the
        # other tiles and double-buffers gathers against the fused dots)
        return (self.eblocks + 5) // 6  # 33

    @property
    def slabs(self):
        out = []
        b = 0
        while b < self.eblocks:
            e = min(b + self.slab_blocks, self.eblocks)
            out.append((b, e))
            b = e
        return out


CFG = Cfg()


def build_kernel(cfg: Cfg):
    """Build + compile the Bacc module. Returns nc."""
    import concourse.bacc as bacc
    import concourse.mybir as mybir
    from concourse import tile

    f32 = mybir.dt.float32
    f16 = mybir.dt.float16
    bf16 = mybir.dt.bfloat16
    i16 = mybir.dt.int16

    D, KB, NSHP, NB = cfg.d, cfg.kb, cfg.nshp, cfg.nblocks
    NTAB, WSH, IC = cfg.ntab, cfg.wsh, cfg.idx_cols
    group = [list(range(cfg.n_cores))]

    nc = bacc.Bacc(
        "TRN2", target_bir_lowering=False, debug=False, num_devices=cfg.n_cores
    )

    zin = nc.dram_tensor("zin", [NSHP, D], bf16, kind="ExternalInput")
    win = nc.dram_tensor("win", [WSH, D], bf16, kind="ExternalInput")
    eidx = nc.dram_tensor("eidx", [16, 2 * IC], i16, kind="ExternalInput")
    scores = nc.dram_tensor("scores", [128, cfg.eblocks], f16, kind="ExternalOutput")
    # Shared-scratchpad AllGather outputs: one copy in chip HBM instead of
    # eight core-local replicas (supported for AllGather with 8 cores).
    wag_out = nc.dram_tensor("wag_out", [D, D], bf16, addr_space="Shared")
    zag_out = nc.dram_tensor("zag_out", [2 * NTAB, D], bf16, addr_space="Shared")

    with tile.TileContext(nc) as tc:
        with (
            tc.tile_pool(name="const", bufs=1) as constp,
            tc.tile_pool(name="dram", bufs=1, space="DRAM") as dramp,
            tc.tile_pool(name="zwsb", bufs=2) as zwp,
            tc.tile_pool(name="rows", bufs=2) as rowsp,
            tc.tile_pool(name="cols", bufs=2) as colsp,
            tc.tile_pool(name="ps", bufs=2, space="PSUM") as psp,
        ):
            # ---- DRAM bounce buffers (collectives can't touch I/O tensors) ----
            # Combined z+zw AllGather: each core contributes [z_c; zw_c]
            # [2*NSHP, D]; output is the interleaved table
            # [z_0; zw_0; z_1; zw_1; ...] that host-side index remap targets.
            wag_in = dramp.tile([WSH, D], bf16, tag="wag_in")
            zag_in = dramp.tile([2 * NSHP, D], bf16, tag="zag_in")

            nc.sync.dma_start(wag_in[:], win.ap())
            nc.sync.dma_start(zag_in[:NSHP, :], zin.ap())

            # ---- collectives (gpsimd, straight-line order) ----
            nc.gpsimd.collective_compute(
                "AllGather",
                mybir.AluOpType.bypass,
                replica_groups=group,
                ins=[wag_in.opt()],
                outs=[wag_out.ap()],
            )

            # ---- SBUF constants ----
            w_sb = constp.tile([128, KB, D], bf16, tag="w")
            nc.sync.dma_start(
                w_sb[:], wag_out.ap().rearrange("(kb p) f -> p kb f", p=128)
            )
            # z^T for the matmul's stationary operand: [128, kb, NSHP]
            zt_sb = constp.tile([128, KB, NSHP], bf16, tag="zt")
            nc.sync.dma_start_transpose(zt_sb[:], zin.ap())
            # edge indices: upload 16-partition wrap, replicate to 128
            idx_sb = constp.tile([128, 2 * IC], i16, tag="idx")
            nc.sync.dma_start(idx_sb[0:16, :], eidx.ap())
            for p in (16, 32, 64):  # replicate 16 -> 128 partitions by doubling
                nc.sync.dma_start(idx_sb[p : 2 * p, :], idx_sb[0:p, :])
            scores_sb = constp.tile([128, cfg.eblocks], f32, tag="scores")
            sig_sb = constp.tile([128, cfg.eblocks], f16, tag="sig")

            # ---- phase 1: ZW_c = z_c @ W for the local node shard ----
            # node blocks in pairs: 8 matmuls share one PSUM->SBUF copy and
            # one DMA (per-instruction dispatch overhead dominates on HW)
            for nb2 in range(NB // 2):
                ps = psp.tile([128, 2, D], f32, tag="ps")
                for h in range(2):
                    for kb in range(KB):
                        nc.tensor.matmul(
                            ps[:, h, :],
                            lhsT=zt_sb[:, kb, (2 * nb2 + h) * 128 : (2 * nb2 + h + 1) * 128],
                            rhs=w_sb[:, kb, :],
                            start=(kb == 0),
                            stop=(kb == KB - 1),
                        )
                zw_t = zwp.tile([128, 2, D], bf16, tag="zwt")
                nc.vector.tensor_copy(zw_t[:], ps[:])
                nc.sync.dma_start(
                    zag_in[NSHP + nb2 * 256 : NSHP + (nb2 + 1) * 256, :].rearrange(
                        "(b p) f -> p b f", p=128
                    ),
                    zw_t[:],
                )

            nc.gpsimd.collective_compute(
                "AllGather",
                mybir.AluOpType.bypass,
                replica_groups=group,
                ins=[zag_in.opt()],
                outs=[zag_out.ap()],
            )

            # ---- phase 2: gathers + per-edge dots ----
            # Real device time here is dominated by per-instruction dispatch
            # overhead, so the dot products are fused: all 196 edge-blocks are
            # processed in 6 slabs, each one giant in-place DVE multiply over
            # [128, nb, 512] plus one tensor_reduce -> [128, nb] score columns
            # (12 instructions instead of 392).
            for t, (b0, b1) in enumerate(cfg.slabs):
                nb = b1 - b0
                ctile = colsp.tile([128, cfg.slab_blocks, D], bf16, tag="ct")
                rtile = rowsp.tile([128, cfg.slab_blocks, D], bf16, tag="rt")
                gstep = nb * 128 if cfg.big_gather else cfg.gchunk
                for c0 in range(0, nb * 128, gstep):
                    G = min(gstep, nb * 128 - c0)
                    off = (b0 * 128 + c0) // 16
                    ob = c0 // 128
                    gb = G // 128
                    nc.gpsimd.dma_gather(
                        ctile[:, ob : ob + gb, :],
                        zag_out.ap(),
                        idx_sb[:, IC + off : IC + off + G // 16],
                        num_idxs=G,
                        num_idxs_reg=G,
                        elem_size=D,
                        single_packet=not cfg.big_gather,
                    )
                    nc.gpsimd.dma_gather(
                        rtile[:, ob : ob + gb, :],
                        zag_out.ap(),
                        idx_sb[:, off : off + G // 16],
                        num_idxs=G,
                        num_idxs_reg=G,
                        elem_size=D,
                        single_packet=not cfg.big_gather,
                    )
                nc.vector.tensor_mul(
                    rtile[:, :nb, :], rtile[:, :nb, :], ctile[:, :nb, :]
                )
                nc.vector.tensor_reduce(
                    scores_sb[:, b0:b1],
                    rtile[:, :nb, :],
                    axis=mybir.AxisListType.X,
                    op=mybir.AluOpType.add,
                )

            # ---- sigmoid + writeback ----
            nc.scalar.activation(
                sig_sb[:], scores_sb[:], mybir.ActivationFunctionType.Sigmoid
            )
            nc.sync.dma_start(scores.ap(), sig_sb[:])

    nc.compile()
    return nc


def _wrap_idx(ids: np.ndarray, cfg: Cfg) -> np.ndarray:
    """int table-row ids [ep_core] -> [16, ep_core//16] int16 in the
    16-partition wrapped layout dma_gather expects."""
    out = np.empty((16, cfg.ep_core // 16), dtype=np.int16)
    off = 0
    for G in cfg.chunks:
        c = ids[off : off + G].reshape(G // 16, 16).T  # [16, G/16]
        out[:, off // 16 : (off + G) // 16] = c
        off += G
    return out


def prep_inputs(z_drug, weight, batch_edges, cfg: Cfg):
    """Host-side layout prep. Returns per-core input maps."""
    import ml_dtypes

    bf16 = ml_dtypes.bfloat16

    z = np.asarray(z_drug, dtype=np.float32)
    w = np.asarray(weight, dtype=np.float32)
    be = np.asarray(batch_edges)

    per_core = []
    for c in range(cfg.n_cores):
        # z shard: nodes [c*1250, (c+1)*1250), padded to 1280 rows
        zsh = np.zeros((cfg.nshp, cfg.d), dtype=bf16)
        zsh[: cfg.nsh] = z[c * cfg.nsh : (c + 1) * cfg.nsh].astype(bf16)
        # W shard: rows [c*64, (c+1)*64)
        wsh = np.ascontiguousarray(
            w[c * cfg.wsh : (c + 1) * cfg.wsh].astype(bf16)
        )
        # edge shard + remap node ids to the padded AllGather table layout
        sl = slice(c * cfg.e_core, (c + 1) * cfg.e_core)
        rids = np.zeros(cfg.ep_core, dtype=np.int64)
        cids = np.zeros(cfg.ep_core, dtype=np.int64)
        rids[: cfg.e_core] = be[0, sl]
        cids[: cfg.e_core] = be[1, sl]
        # combined table layout: [z_0; zw_0; z_1; zw_1; ...], stride 2*nshp
        rids = (rids // cfg.nsh) * (2 * cfg.nshp) + cfg.nshp + rids % cfg.nsh
        cids = (cids // cfg.nsh) * (2 * cfg.nshp) + cids % cfg.nsh
        eidx = np.concatenate(
            [_wrap_idx(rids, cfg), _wrap_idx(cids, cfg)], axis=1
        )
        per_core.append({"zin": zsh, "win": wsh, "eidx": eidx})
    return per_core


_NC_CACHE = {}


def get_nc(cfg: Cfg):
    key = (cfg.gchunk, cfg.big_gather)
    if key not in _NC_CACHE:
        _NC_CACHE[key] = build_kernel(cfg)
    return _NC_CACHE[key]


def _unshard(results, cfg: Cfg) -> np.ndarray:
    parts = []
    for c in range(cfg.n_cores):
        raw = results[c]["scores"]  # [128, eblocks], edge i at [i%128, i//128]
        parts.append(raw.T.reshape(-1)[: cfg.e_core])
    return np.concatenate(parts).astype(np.float32)


def run(z_drug, weight, batch_edges, cfg: Cfg, repeats: int = 1):
    """Returns (scores[200000] f32, [wall seconds per call])."""
    import time

    from concourse.bass_utils import run_bass_kernel_spmd

    nc = get_nc(cfg)
    in_maps = prep_inputs(z_drug, weight, batch_edges, cfg)
    walls = []
    res = None
    for _ in range(max(1, repeats)):
        t0 = time.perf_counter()
        try:
            res = run_bass_kernel_spmd(
                nc, in_maps, core_ids=list(range(cfg.n_cores))
            )
        except Exception:
            if res is not None:
                break  # keep earlier good result; a repeat run hiccupped
            time.sleep(30)
            res = run_bass_kernel_spmd(
                nc, in_maps, core_ids=list(range(cfg.n_cores))
            )
        walls.append(time.perf_counter() - t0)
    return _unshard(res.results, cfg), walls


def kernel(z_drug, weight, batch_edges):
    out, _ = run(z_drug, weight, batch_edges, CFG)
    return out


# revision 36
# speedup vs baseline: 1.7969x; 1.0709x over previous
"""Bilinear decoder kernel for Trainium2 (8 NeuronCores).

score_e = sigmoid(z[row_e] @ W @ z[col_e])  for 200k edges, d=512.

Strategy (host->device transfer over axon is ~40MB/s, so uploads are
sharded and the tables are rebuilt on-device with AllGathers):
  - Upload per core (~1.5MB vs ~41MB for a replicated-f32 design):
    z shard [1280,512] bf16 (1/8 of nodes), W shard [64,512] bf16
    (1/8 of rows), edge indices [16, 2*1568] int16.
  - Device: AllGather W (tiny) -> full W in SBUF. Load z^T via
    dma_start_transpose, matmul ZW_c = z_c @ W for the local 1280-node
    shard (tensor engine, bf16). One combined AllGather of [z_c; zw_c]
    [2560,512] -> interleaved table [z_0; zw_0; z_1; zw_1; ...]
    [20480,512] bf16 in a Shared-addr-space scratchpad (one HBM copy for
    all 8 cores on the chip).
  - Gather ZW[row_e] and Z[col_e] rows from the shared table via
    dma_gather (512-row chunks) into 6 slabs of 33 edge-blocks; per-edge
    dots are FUSED per slab (one in-place DVE tensor_mul + one
    tensor_reduce over [128,33,512]) because real HW pays ~100us
    dispatch overhead per instruction; sigmoid on ACT, f16 scores out.
  - Edges sharded 25000/core; node ids are remapped on host to the
    interleaved table layout (col c -> (c//1250)*2560 + c%1250, row r
    adds +1280).

Host-side work is layout-only: bf16 casts, shard slicing, index
wrap/remap, output unshard.
"""

import sys

if "/opt/trn_rl_repo" not in sys.path:
    sys.path.insert(0, "/opt/trn_rl_repo")

from dataclasses import dataclass

import numpy as np


@dataclass(frozen=True)
class Cfg:
    n_cores: int = 8
    d: int = 512              # embedding dim
    n_nodes: int = 10000      # node table rows
    e_total: int = 200000     # total edges
    gchunk: int = 512         # edges per dma_gather (SDMA packet limit:
    #                           512 rows = 32 descriptors/engine works,
    #                           1024+ faults the exec unit with
    #                           single_packet=True)
    big_gather: bool = False  # one dma_gather per slab (single_packet=False)
    gq2: bool = False         # row/col gathers on separate SWDGE queues

    @property
    def kb(self):
        return self.d // 128  # 4

    @property
    def nsh(self):
        return self.n_nodes // self.n_cores  # 1250 nodes per core

    @property
    def nshp(self):
        return ((self.nsh + 127) // 128) * 128  # 1280 padded

    @property
    def nblocks(self):
        return self.nshp // 128  # 10

    @property
    def ntab(self):
        return self.nshp * self.n_cores  # 10240 table rows

    @property
    def wsh(self):
        return self.d // self.n_cores  # 64 W rows per core

    @property
    def e_core(self):
        return self.e_total // self.n_cores  # 25000

    @property
    def ep_core(self):
        return ((self.e_core + 127) // 128) * 128  # 25088

    @property
    def eblocks(self):
        return self.ep_core // 128  # 196

    @property
    def idx_cols(self):
        return self.ep_core // 16  # 1568

    @property
    def chunks(self):
        out = []
        left = self.ep_core
        while left > 0:
            c = min(self.gchunk, left)
            out.append(c)
            left -= c
        return out

    @property
    def slab_blocks(self):
        # 196 edge-blocks in 6 slabs; slab tiles are [128, 33, 512] bf16
        # (33KB/partition, x2 tables x2 bufs = 132KB, fits SBUF beside the
        # other tiles and double-buffers gathers against the fused dots)
        return (self.eblocks + 5) // 6  # 33

    @property
    def slabs(self):
        out = []
        b = 0
        while b < self.eblocks:
            e = min(b + self.slab_blocks, self.eblocks)
            out.append((b, e))
            b = e
        return out


CFG = Cfg()


def build_kernel(cfg: Cfg):
    """Build + compile the Bacc module. Returns nc."""
    import concourse.bacc as bacc
    import concourse.mybir as mybir
    from concourse import tile

    f32 = mybir.dt.float32
    f16 = mybir.dt.float16
    bf16 = mybir.dt.bfloat16
    i16 = mybir.dt.int16

    D, KB, NSHP, NB = cfg.d, cfg.kb, cfg.nshp, cfg.nblocks
    NTAB, WSH, IC = cfg.ntab, cfg.wsh, cfg.idx_cols
    group = [list(range(cfg.n_cores))]

    nc = bacc.Bacc(
        "TRN2",
        target_bir_lowering=False,
        debug=False,
        num_devices=cfg.n_cores,
        num_swdge_queues=2 if cfg.gq2 else 1,
    )

    zin = nc.dram_tensor("zin", [NSHP, D], bf16, kind="ExternalInput")
    win = nc.dram_tensor("win", [WSH, D], bf16, kind="ExternalInput")
    eidx = nc.dram_tensor("eidx", [16, 2 * IC], i16, kind="ExternalInput")
    scores = nc.dram_tensor("scores", [128, cfg.eblocks], f16, kind="ExternalOutput")
    # Shared-scratchpad AllGather outputs: one copy in chip HBM instead of
    # eight core-local replicas (supported for AllGather with 8 cores).
    wag_out = nc.dram_tensor("wag_out", [D, D], bf16, addr_space="Shared")
    zag_out = nc.dram_tensor("zag_out", [2 * NTAB, D], bf16, addr_space="Shared")

    with tile.TileContext(nc) as tc:
        with (
            tc.tile_pool(name="const", bufs=1) as constp,
            tc.tile_pool(name="dram", bufs=1, space="DRAM") as dramp,
            tc.tile_pool(name="zwsb", bufs=2) as zwp,
            tc.tile_pool(name="rows", bufs=2) as rowsp,
            tc.tile_pool(name="cols", bufs=2) as colsp,
            tc.tile_pool(name="ps", bufs=2, space="PSUM") as psp,
        ):
            # ---- DRAM bounce buffers (collectives can't touch I/O tensors) ----
            # Combined z+zw AllGather: each core contributes [z_c; zw_c]
            # [2*NSHP, D]; output is the interleaved table
            # [z_0; zw_0; z_1; zw_1; ...] that host-side index remap targets.
            wag_in = dramp.tile([WSH, D], bf16, tag="wag_in")
            zag_in = dramp.tile([2 * NSHP, D], bf16, tag="zag_in")

            nc.sync.dma_start(wag_in[:], win.ap())
            nc.sync.dma_start(zag_in[:NSHP, :], zin.ap())

            # ---- collectives (gpsimd, straight-line order) ----
            nc.gpsimd.collective_compute(
                "AllGather",
                mybir.AluOpType.bypass,
                replica_groups=group,
                ins=[wag_in.opt()],
                outs=[wag_out.ap()],
            )

            # ---- SBUF constants ----
            w_sb = constp.tile([128, KB, D], bf16, tag="w")
            nc.sync.dma_start(
                w_sb[:], wag_out.ap().rearrange("(kb p) f -> p kb f", p=128)
            )
            # z^T for the matmul's stationary operand: [128, kb, NSHP]
            zt_sb = constp.tile([128, KB, NSHP], bf16, tag="zt")
            nc.sync.dma_start_transpose(zt_sb[:], zin.ap())
            # edge indices: upload 16-partition wrap, replicate to 128
            idx_sb = constp.tile([128, 2 * IC], i16, tag="idx")
            nc.sync.dma_start(idx_sb[0:16, :], eidx.ap())
            for p in (16, 32, 64):  # replicate 16 -> 128 partitions by doubling
                nc.sync.dma_start(idx_sb[p : 2 * p, :], idx_sb[0:p, :])
            scores_sb = constp.tile([128, cfg.eblocks], f32, tag="scores")
            sig_sb = constp.tile([128, cfg.eblocks], f16, tag="sig")

            # ---- phase 1: ZW_c = z_c @ W for the local node shard ----
            # node blocks in pairs: 8 matmuls share one PSUM->SBUF copy and
            # one DMA (per-instruction dispatch overhead dominates on HW)
            for nb2 in range(NB // 2):
                ps = psp.tile([128, 2, D], f32, tag="ps")
                for h in range(2):
                    for kb in range(KB):
                        nc.tensor.matmul(
                            ps[:, h, :],
                            lhsT=zt_sb[:, kb, (2 * nb2 + h) * 128 : (2 * nb2 + h + 1) * 128],
                            rhs=w_sb[:, kb, :],
                            start=(kb == 0),
                            stop=(kb == KB - 1),
                        )
                zw_t = zwp.tile([128, 2, D], bf16, tag="zwt")
                nc.vector.tensor_copy(zw_t[:], ps[:])
                nc.sync.dma_start(
                    zag_in[NSHP + nb2 * 256 : NSHP + (nb2 + 1) * 256, :].rearrange(
                        "(b p) f -> p b f", p=128
                    ),
                    zw_t[:],
                )

            nc.gpsimd.collective_compute(
                "AllGather",
                mybir.AluOpType.bypass,
                replica_groups=group,
                ins=[zag_in.opt()],
                outs=[zag_out.ap()],
            )

            # ---- phase 2: gathers + per-edge dots ----
            # Real device time here is dominated by per-instruction dispatch
            # overhead, so the dot products are fused: all 196 edge-blocks are
            # processed in 6 slabs, each one giant in-place DVE multiply over
            # [128, nb, 512] plus one tensor_reduce -> [128, nb] score columns
            # (12 instructions instead of 392).
            for t, (b0, b1) in enumerate(cfg.slabs):
                nb = b1 - b0
                ctile = colsp.tile([128, cfg.slab_blocks, D], bf16, tag="ct")
                rtile = rowsp.tile([128, cfg.slab_blocks, D], bf16, tag="rt")
                gstep = nb * 128 if cfg.big_gather else cfg.gchunk
                for c0 in range(0, nb * 128, gstep):
                    G = min(gstep, nb * 128 - c0)
                    off = (b0 * 128 + c0) // 16
                    ob = c0 // 128
                    gb = G // 128
                    nc.gpsimd.dma_gather(
                        ctile[:, ob : ob + gb, :],
                        zag_out.ap(),
                        idx_sb[:, IC + off : IC + off + G // 16],
                        num_idxs=G,
                        num_idxs_reg=G,
                        elem_size=D,
                        single_packet=not cfg.big_gather,
                    )
                    nc.gpsimd.dma_gather(
                        rtile[:, ob : ob + gb, :],
                        zag_out.ap(),
                        idx_sb[:, off : off + G // 16],
                        num_idxs=G,
                        num_idxs_reg=G,
                        elem_size=D,
                        single_packet=not cfg.big_gather,
                        queue_num=1 if cfg.gq2 else 0,
                    )
                nc.vector.tensor_mul(
                    rtile[:, :nb, :], rtile[:, :nb, :], ctile[:, :nb, :]
                )
                nc.vector.tensor_reduce(
                    scores_sb[:, b0:b1],
                    rtile[:, :nb, :],
                    axis=mybir.AxisListType.X,
                    op=mybir.AluOpType.add,
                )

            # ---- sigmoid + writeback ----
            nc.scalar.activation(
                sig_sb[:], scores_sb[:], mybir.ActivationFunctionType.Sigmoid
            )
            nc.sync.dma_start(scores.ap(), sig_sb[:])

    nc.compile()
    return nc


def _wrap_idx(ids: np.ndarray, cfg: Cfg) -> np.ndarray:
    """int table-row ids [ep_core] -> [16, ep_core//16] int16 in the
    16-partition wrapped layout dma_gather expects (id j at [j%16, j//16];
    per-chunk wrapping is equivalent to this global wrap whenever every
    gather range is 16-aligned, which gchunk=512 guarantees)."""
    return np.ascontiguousarray(ids.reshape(-1, 16).astype(np.int16).T)


def prep_inputs(z_drug, weight, batch_edges, cfg: Cfg):
    """Host-side layout prep. Returns per-core input maps."""
    import ml_dtypes

    bf16 = ml_dtypes.bfloat16

    z = np.asarray(z_drug, dtype=np.float32)
    w = np.asarray(weight, dtype=np.float32)
    be = np.asarray(batch_edges)

    per_core = []
    for c in range(cfg.n_cores):
        # z shard: nodes [c*1250, (c+1)*1250), padded to 1280 rows
        zsh = np.zeros((cfg.nshp, cfg.d), dtype=bf16)
        zsh[: cfg.nsh] = z[c * cfg.nsh : (c + 1) * cfg.nsh].astype(bf16)
        # W shard: rows [c*64, (c+1)*64)
        wsh = np.ascontiguousarray(
            w[c * cfg.wsh : (c + 1) * cfg.wsh].astype(bf16)
        )
        # edge shard + remap node ids to the padded AllGather table layout
        sl = slice(c * cfg.e_core, (c + 1) * cfg.e_core)
        rids = np.zeros(cfg.ep_core, dtype=np.int64)
        cids = np.zeros(cfg.ep_core, dtype=np.int64)
        rids[: cfg.e_core] = be[0, sl]
        cids[: cfg.e_core] = be[1, sl]
        # combined table layout: [z_0; zw_0; z_1; zw_1; ...], stride 2*nshp
        rids = (rids // cfg.nsh) * (2 * cfg.nshp) + cfg.nshp + rids % cfg.nsh
        cids = (cids // cfg.nsh) * (2 * cfg.nshp) + cids % cfg.nsh
        eidx = np.concatenate(
            [_wrap_idx(rids, cfg), _wrap_idx(cids, cfg)], axis=1
        )
        per_core.append({"zin": zsh, "win": wsh, "eidx": eidx})
    return per_core


_NC_CACHE = {}


def get_nc(cfg: Cfg):
    key = (cfg.gchunk, cfg.big_gather, cfg.gq2)
    if key not in _NC_CACHE:
        _NC_CACHE[key] = build_kernel(cfg)
    return _NC_CACHE[key]


class CachedRunner:
    """One jitted shard_map executable per module, reused across calls.

    The stock run_bass_kernel_spmd path rebuilds closures and re-traces
    jax.jit on EVERY call (~170ms with this NEFF embedded in the HLO).
    Caching the jitted fn and feeding plain numpy args avoids that; unlike
    the device-resident-input Runner pattern, numpy args do not desync the
    axon mesh (verified over interleaved calls on HW).
    """

    def __init__(self, nc, n_cores: int):
        import jax
        import concourse.mybir as mybir
        from concourse import bass2jax
        from concourse.bass2jax import _bass_exec_p, partition_id_tensor
        from jax.experimental.shard_map import shard_map
        from jax.sharding import Mesh, PartitionSpec

        bass2jax.install_neuronx_cc_hook()
        self.n_cores = n_cores

        in_names, out_names, out_avals, zero_outs = [], [], [], []
        for alloc in nc.m.functions[0].allocations:
            if not isinstance(alloc, mybir.MemoryLocationSet):
                continue
            name = alloc.memorylocations[0].name
            if alloc.kind == "ExternalInput":
                in_names.append(name)
            elif alloc.kind == "ExternalOutput":
                out_names.append(name)
                shape = tuple(alloc.tensor_shape)
                dtype = mybir.dt.np(alloc.dtype)
                out_avals.append(jax.core.ShapedArray(shape, dtype))
                zero_outs.append(
                    np.zeros((n_cores * shape[0], *shape[1:]), dtype)
                )
        partition_name = (
            nc.partition_id_tensor.name if nc.partition_id_tensor else None
        )
        if partition_name is not None:
            in_names.remove(partition_name)
        full_in_names = in_names + out_names
        if partition_name is not None:
            full_in_names.append(partition_name)
        self.in_names = in_names
        self.out_names = out_names
        self.out_avals = out_avals
        self.zero_outs = zero_outs

        def _body(*args):
            operands = list(args)
            if partition_name is not None:
                operands.append(partition_id_tensor())
            outs = _bass_exec_p.bind(
                *operands,
                out_avals=tuple(out_avals),
                in_names=tuple(full_in_names),
                out_names=tuple(out_names),
                lowering_input_output_aliases=(),
                sim_require_finite=True,
                sim_require_nnan=True,
                nc=nc,
            )
            return tuple(outs)

        devices = jax.devices()[:n_cores]
        mesh = Mesh(np.asarray(devices), ("core",))
        n_args = len(in_names) + len(out_names)
        self.fn = jax.jit(
            shard_map(
                _body,
                mesh=mesh,
                in_specs=(PartitionSpec("core"),) * n_args,
                out_specs=(PartitionSpec("core"),) * len(out_names),
                check_rep=False,
            ),
            keep_unused=True,
        )

    def __call__(self, in_maps):
        concat = [
            np.concatenate(
                [np.asarray(in_maps[c][name]) for c in range(self.n_cores)],
                axis=0,
            )
            for name in self.in_names
        ]
        out_arrs = self.fn(*concat, *self.zero_outs)
        return [
            {
                name: np.asarray(out_arrs[i]).reshape(
                    self.n_cores, *self.out_avals[i].shape
                )[c]
                for i, name in enumerate(self.out_names)
            }
            for c in range(self.n_cores)
        ]


_RUNNER_CACHE = {}


def get_runner(cfg: Cfg) -> CachedRunner:
    key = (cfg.gchunk, cfg.big_gather, cfg.gq2)
    if key not in _RUNNER_CACHE:
        _RUNNER_CACHE[key] = CachedRunner(get_nc(cfg), cfg.n_cores)
    return _RUNNER_CACHE[key]


def _unshard(results, cfg: Cfg) -> np.ndarray:
    parts = []
    for c in range(cfg.n_cores):
        raw = results[c]["scores"]  # [128, eblocks], edge i at [i%128, i//128]
        parts.append(raw.T.reshape(-1)[: cfg.e_core])
    return np.concatenate(parts).astype(np.float32)


def run(z_drug, weight, batch_edges, cfg: Cfg, repeats: int = 1):
    """Returns (scores[200000] f32, [wall seconds per call])."""
    import time

    in_maps = prep_inputs(z_drug, weight, batch_edges, cfg)
    walls = []
    results = None
    for _ in range(max(1, repeats)):
        t0 = time.perf_counter()
        try:
            results = get_runner(cfg)(in_maps)
        except Exception:
            if results is not None:
                break  # keep earlier good result; a repeat run hiccupped
            # fall back to the slower but battle-tested per-call path
            from concourse.bass_utils import run_bass_kernel_spmd

            time.sleep(30)
            results = run_bass_kernel_spmd(
                get_nc(cfg), in_maps, core_ids=list(range(cfg.n_cores))
            ).results
        walls.append(time.perf_counter() - t0)
    return _unshard(results, cfg), walls


def kernel(z_drug, weight, batch_edges):
    out, _ = run(z_drug, weight, batch_edges, CFG)
    return out


# revision 49
# speedup vs baseline: 1.9452x; 1.0825x over previous
"""Bilinear decoder kernel for Trainium2 (8 NeuronCores).

score_e = sigmoid(z[row_e] @ W @ z[col_e])  for 200k edges, d=512.

Strategy (host->device transfer over axon is ~40MB/s, so uploads are
sharded and the tables are rebuilt on-device with AllGathers):
  - Upload per core (~1.5MB vs ~41MB for a replicated-f32 design):
    z shard [1280,512] bf16 (1/8 of nodes), W shard [64,512] bf16
    (1/8 of rows), edge indices [16, 2*1568] int16.
  - Device: AllGather W (tiny) -> full W in SBUF. Load z^T via
    dma_start_transpose, matmul ZW_c = z_c @ W for the local 1280-node
    shard (tensor engine, bf16). One combined AllGather of [z_c; zw_c]
    [2560,512] -> interleaved table [z_0; zw_0; z_1; zw_1; ...]
    [20480,512] bf16 in a Shared-addr-space scratchpad (one HBM copy for
    all 8 cores on the chip).
  - Gather ZW[row_e] and Z[col_e] rows from the shared table via
    dma_gather (512-row chunks) into 6 slabs of 33 edge-blocks; per-edge
    dots are FUSED per slab (one in-place DVE tensor_mul + one
    tensor_reduce over [128,33,512]) because real HW pays ~100us
    dispatch overhead per instruction; sigmoid on ACT, f16 scores out.
  - Edges sharded 25000/core; node ids are remapped on host to the
    interleaved table layout (col c -> (c//1250)*2560 + c%1250, row r
    adds +1280).

Host-side work is layout-only: bf16 casts, shard slicing, index
wrap/remap, output unshard.
"""

import sys

if "/opt/trn_rl_repo" not in sys.path:
    sys.path.insert(0, "/opt/trn_rl_repo")

from dataclasses import dataclass

import numpy as np


@dataclass(frozen=True)
class Cfg:
    n_cores: int = 8
    d: int = 512              # embedding dim
    n_nodes: int = 10000      # node table rows
    e_total: int = 200000     # total edges
    gchunk: int = 512         # edges per dma_gather (SDMA packet limit:
    #                           512 rows = 32 descriptors/engine works,
    #                           1024+ faults the exec unit with
    #                           single_packet=True)
    big_gather: bool = False  # one dma_gather per slab (single_packet=False)
    gq2: bool = False         # row/col gathers on separate SWDGE queues
    z12: bool = False         # 12-bit z upload (int8 + packed int4 residual);
    #                           passes CoreSim (rel 7.9e-3) but its NEFF still
    #                           fails HW compile in this env — keep off
    sA: float = 0.042         # int8 scale (clip at 5.33 sigma)

    @property
    def sB(self):
        return self.sA / 14.0  # int4 residual scale

    @property
    def kb(self):
        return self.d // 128  # 4

    @property
    def nsh(self):
        return self.n_nodes // self.n_cores  # 1250 nodes per core

    @property
    def nshp(self):
        return ((self.nsh + 127) // 128) * 128  # 1280 padded

    @property
    def nblocks(self):
        return self.nshp // 128  # 10

    @property
    def ntab(self):
        return self.nshp * self.n_cores  # 10240 table rows

    @property
    def wsh(self):
        return self.d // self.n_cores  # 64 W rows per core

    @property
    def e_core(self):
        return self.e_total // self.n_cores  # 25000

    @property
    def ep_core(self):
        return ((self.e_core + 127) // 128) * 128  # 25088

    @property
    def eblocks(self):
        return self.ep_core // 128  # 196

    @property
    def idx_cols(self):
        return self.ep_core // 16  # 1568

    @property
    def chunks(self):
        out = []
        left = self.ep_core
        while left > 0:
            c = min(self.gchunk, left)
            out.append(c)
            left -= c
        return out

    @property
    def slab_blocks(self):
        # 196 edge-blocks in 6 slabs; slab tiles are [128, 33, 512] bf16
        # (33KB/partition, x2 tables x2 bufs = 132KB, fits SBUF beside the
        # other tiles and double-buffers gathers against the fused dots)
        return (self.eblocks + 5) // 6  # 33

    @property
    def slabs(self):
        out = []
        b = 0
        while b < self.eblocks:
            e = min(b + self.slab_blocks, self.eblocks)
            out.append((b, e))
            b = e
        return out


CFG = Cfg()


def build_kernel(cfg: Cfg):
    """Build + compile the Bacc module. Returns nc."""
    import concourse.bacc as bacc
    import concourse.mybir as mybir
    from concourse import tile

    f32 = mybir.dt.float32
    f16 = mybir.dt.float16
    bf16 = mybir.dt.bfloat16
    i16 = mybir.dt.int16

    D, KB, NSHP, NB = cfg.d, cfg.kb, cfg.nshp, cfg.nblocks
    NTAB, WSH, IC = cfg.ntab, cfg.wsh, cfg.idx_cols
    group = [list(range(cfg.n_cores))]
    AL = mybir.AluOpType

    nc = bacc.Bacc(
        "TRN2",
        target_bir_lowering=False,
        debug=False,
        num_devices=cfg.n_cores,
        num_swdge_queues=2 if cfg.gq2 else 1,
    )

    i8 = mybir.dt.int8
    if cfg.z12:
        zin8 = nc.dram_tensor("zin8", [NSHP, D], i8, kind="ExternalInput")
        zn4 = nc.dram_tensor("zn4", [NSHP, D // 2], i8, kind="ExternalInput")
    else:
        zin = nc.dram_tensor("zin", [NSHP, D], bf16, kind="ExternalInput")
    win = nc.dram_tensor("win", [WSH, D], bf16, kind="ExternalInput")
    eidx = nc.dram_tensor("eidx", [16, 2 * IC], i16, kind="ExternalInput")
    scores = nc.dram_tensor("scores", [128, cfg.eblocks], f16, kind="ExternalOutput")
    # Shared-scratchpad AllGather outputs: one copy in chip HBM instead of
    # eight core-local replicas (supported for AllGather with 8 cores).
    wag_out = nc.dram_tensor("wag_out", [D, D], bf16, addr_space="Shared")
    zag_out = nc.dram_tensor("zag_out", [2 * NTAB, D], bf16, addr_space="Shared")

    with tile.TileContext(nc) as tc:
        with (
            tc.tile_pool(name="const", bufs=1) as constp,
            tc.tile_pool(name="dram", bufs=1, space="DRAM") as dramp,
            tc.tile_pool(name="zwsb", bufs=2) as zwp,
            tc.tile_pool(name="rows", bufs=2) as rowsp,
            tc.tile_pool(name="cols", bufs=2) as colsp,
            tc.tile_pool(name="ps", bufs=2, space="PSUM") as psp,
        ):
            # ---- DRAM bounce buffers (collectives can't touch I/O tensors) ----
            # Combined z+zw AllGather: each core contributes [z_c; zw_c]
            # [2*NSHP, D]; output is the interleaved table
            # [z_0; zw_0; z_1; zw_1; ...] that host-side index remap targets.
            wag_in = dramp.tile([WSH, D], bf16, tag="wag_in")
            zag_in = dramp.tile([2 * NSHP, D], bf16, tag="zag_in")

            nc.sync.dma_start(wag_in[:], win.ap())
            if cfg.z12:
                # unpack 12-bit z (int8 coarse + int4 residual halves) to bf16
                a3 = constp.tile([128, NB, D], i8, tag="a3")
                nc.sync.dma_start(
                    a3[:], zin8.ap().rearrange("(b p) f -> p b f", p=128)
                )
                n3 = constp.tile([128, NB, D // 2], i8, tag="n3")
                nc.sync.dma_start(
                    n3[:], zn4.ap().rearrange("(b p) f -> p b f", p=128)
                )
                zd = constp.tile([128, NB, D], bf16, tag="zd")
                nc.vector.tensor_scalar(zd[:], a3[:], cfg.sA, None, AL.mult)
                ev8 = constp.tile([128, NB, D // 2], i8, tag="ev8")
                # walrus rejects chained bitwise+arith ops in one tensor_scalar
                nc.vector.tensor_scalar(ev8[:], n3[:], 15, None, AL.bitwise_and)
                nc.vector.tensor_scalar(ev8[:], ev8[:], -8, None, AL.add)
                # shifts fail walrus's tensor_scalar_shift_chk; (n & -16) is
                # the high nibble *16 sign-safely, folded into the scale below
                od8 = constp.tile([128, NB, D // 2], i8, tag="od8")
                nc.vector.tensor_scalar(od8[:], n3[:], -16, None, AL.bitwise_and)
                evf = constp.tile([128, NB, D // 2], bf16, tag="evf")
                nc.vector.tensor_scalar(evf[:], ev8[:], cfg.sB, None, AL.mult)
                odf = constp.tile([128, NB, D // 2], bf16, tag="odf")
                nc.vector.tensor_scalar(
                    odf[:], od8[:], cfg.sB / 16.0, None, AL.mult
                )
                nc.vector.tensor_tensor(
                    zd[:, :, : D // 2], zd[:, :, : D // 2], evf[:], op=AL.add
                )
                nc.vector.tensor_tensor(
                    zd[:, :, D // 2 :], zd[:, :, D // 2 :], odf[:], op=AL.add
                )
                nc.sync.dma_start(
                    zag_in[:NSHP, :].rearrange("(b p) f -> p b f", p=128),
                    zd[:],
                )
            else:
                nc.sync.dma_start(zag_in[:NSHP, :], zin.ap())

            # ---- collectives (gpsimd, straight-line order) ----
            nc.gpsimd.collective_compute(
                "AllGather",
                mybir.AluOpType.bypass,
                replica_groups=group,
                ins=[wag_in.opt()],
                outs=[wag_out.ap()],
            )

            # ---- SBUF constants ----
            w_sb = constp.tile([128, KB, D], bf16, tag="w")
            nc.sync.dma_start(
                w_sb[:], wag_out.ap().rearrange("(kb p) f -> p kb f", p=128)
            )
            # z^T for the matmul's stationary operand: [128, kb, NSHP]
            zt_sb = constp.tile([128, KB, NSHP], bf16, tag="zt")
            if cfg.z12:
                nc.sync.dma_start_transpose(zt_sb[:], zag_in[:NSHP, :])
            else:
                nc.sync.dma_start_transpose(zt_sb[:], zin.ap())
            # edge indices: upload 16-partition wrap, replicate to 128
            idx_sb = constp.tile([128, 2 * IC], i16, tag="idx")
            nc.sync.dma_start(idx_sb[0:16, :], eidx.ap())
            for p in (16, 32, 64):  # replicate 16 -> 128 partitions by doubling
                nc.sync.dma_start(idx_sb[p : 2 * p, :], idx_sb[0:p, :])
            scores_sb = constp.tile([128, cfg.eblocks], f32, tag="scores")
            sig_sb = constp.tile([128, cfg.eblocks], f16, tag="sig")

            # ---- phase 1: ZW_c = z_c @ W for the local node shard ----
            # node blocks in pairs: 8 matmuls share one PSUM->SBUF copy and
            # one DMA (per-instruction dispatch overhead dominates on HW)
            for nb2 in range(NB // 2):
                ps = psp.tile([128, 2, D], f32, tag="ps")
                for h in range(2):
                    for kb in range(KB):
                        nc.tensor.matmul(
                            ps[:, h, :],
                            lhsT=zt_sb[:, kb, (2 * nb2 + h) * 128 : (2 * nb2 + h + 1) * 128],
                            rhs=w_sb[:, kb, :],
                            start=(kb == 0),
                            stop=(kb == KB - 1),
                        )
                zw_t = zwp.tile([128, 2, D], bf16, tag="zwt")
                nc.vector.tensor_copy(zw_t[:], ps[:])
                nc.sync.dma_start(
                    zag_in[NSHP + nb2 * 256 : NSHP + (nb2 + 1) * 256, :].rearrange(
                        "(b p) f -> p b f", p=128
                    ),
                    zw_t[:],
                )

            nc.gpsimd.collective_compute(
                "AllGather",
                mybir.AluOpType.bypass,
                replica_groups=group,
                ins=[zag_in.opt()],
                outs=[zag_out.ap()],
            )

            # ---- phase 2: gathers + per-edge dots ----
            # Real device time here is dominated by per-instruction dispatch
            # overhead, so the dot products are fused: all 196 edge-blocks are
            # processed in 6 slabs, each one giant in-place DVE multiply over
            # [128, nb, 512] plus one tensor_reduce -> [128, nb] score columns
            # (12 instructions instead of 392).
            for t, (b0, b1) in enumerate(cfg.slabs):
                nb = b1 - b0
                ctile = colsp.tile([128, cfg.slab_blocks, D], bf16, tag="ct")
                rtile = rowsp.tile([128, cfg.slab_blocks, D], bf16, tag="rt")
                gstep = nb * 128 if cfg.big_gather else cfg.gchunk
                for c0 in range(0, nb * 128, gstep):
                    G = min(gstep, nb * 128 - c0)
                    off = (b0 * 128 + c0) // 16
                    ob = c0 // 128
                    gb = G // 128
                    nc.gpsimd.dma_gather(
                        ctile[:, ob : ob + gb, :],
                        zag_out.ap(),
                        idx_sb[:, IC + off : IC + off + G // 16],
                        num_idxs=G,
                        num_idxs_reg=G,
                        elem_size=D,
                        single_packet=not cfg.big_gather,
                    )
                    nc.gpsimd.dma_gather(
                        rtile[:, ob : ob + gb, :],
                        zag_out.ap(),
                        idx_sb[:, off : off + G // 16],
                        num_idxs=G,
                        num_idxs_reg=G,
                        elem_size=D,
                        single_packet=not cfg.big_gather,
                        queue_num=1 if cfg.gq2 else 0,
                    )
                nc.vector.tensor_mul(
                    rtile[:, :nb, :], rtile[:, :nb, :], ctile[:, :nb, :]
                )
                nc.vector.tensor_reduce(
                    scores_sb[:, b0:b1],
                    rtile[:, :nb, :],
                    axis=mybir.AxisListType.X,
                    op=mybir.AluOpType.add,
                )

            # ---- sigmoid + writeback ----
            nc.scalar.activation(
                sig_sb[:], scores_sb[:], mybir.ActivationFunctionType.Sigmoid
            )
            nc.sync.dma_start(scores.ap(), sig_sb[:])

    nc.compile()
    return nc


def _wrap_idx(ids: np.ndarray, cfg: Cfg) -> np.ndarray:
    """int table-row ids [ep_core] -> [16, ep_core//16] int16 in the
    16-partition wrapped layout dma_gather expects (id j at [j%16, j//16];
    per-chunk wrapping is equivalent to this global wrap whenever every
    gather range is 16-aligned, which gchunk=512 guarantees)."""
    return np.ascontiguousarray(ids.reshape(-1, 16).astype(np.int16).T)


def prep_inputs(z_drug, weight, batch_edges, cfg: Cfg):
    """Host-side layout prep. Returns per-core input maps."""
    import ml_dtypes

    bf16 = ml_dtypes.bfloat16

    z = np.asarray(z_drug, dtype=np.float32)
    w = np.asarray(weight, dtype=np.float32)
    be = np.asarray(batch_edges)

    per_core = []
    for c in range(cfg.n_cores):
        # z shard: nodes [c*1250, (c+1)*1250), padded to 1280 rows
        zf = z[c * cfg.nsh : (c + 1) * cfg.nsh]
        if cfg.z12:
            # 12-bit: int8 coarse + int4 residual, residual halves packed
            # as low nibble = elem f (+8 offset), high nibble = elem f+256
            A = np.clip(np.round(zf / cfg.sA), -127, 127).astype(np.int8)
            R = zf - A.astype(np.float32) * cfg.sA
            B = np.clip(np.round(R / cfg.sB), -8, 7).astype(np.int8)
            a8 = np.zeros((cfg.nshp, cfg.d), np.int8)
            a8[: cfg.nsh] = A
            hd = cfg.d // 2
            packed = (
                ((B[:, hd:].astype(np.uint8) & 0xF) << 4)
                | ((B[:, :hd].astype(np.int16) + 8).astype(np.uint8) & 0xF)
            ).astype(np.uint8)
            n4 = np.full((cfg.nshp, hd), 8, np.uint8)  # pad decodes to 0
            n4[: cfg.nsh] = packed
            n4 = n4.view(np.int8)
        else:
            zsh = np.zeros((cfg.nshp, cfg.d), dtype=bf16)
            zsh[: cfg.nsh] = zf.astype(bf16)
        # W shard: rows [c*64, (c+1)*64)
        wsh = np.ascontiguousarray(
            w[c * cfg.wsh : (c + 1) * cfg.wsh].astype(bf16)
        )
        # edge shard + remap node ids to the padded AllGather table layout
        sl = slice(c * cfg.e_core, (c + 1) * cfg.e_core)
        rids = np.zeros(cfg.ep_core, dtype=np.int64)
        cids = np.zeros(cfg.ep_core, dtype=np.int64)
        rids[: cfg.e_core] = be[0, sl]
        cids[: cfg.e_core] = be[1, sl]
        # combined table layout: [z_0; zw_0; z_1; zw_1; ...], stride 2*nshp
        rids = (rids // cfg.nsh) * (2 * cfg.nshp) + cfg.nshp + rids % cfg.nsh
        cids = (cids // cfg.nsh) * (2 * cfg.nshp) + cids % cfg.nsh
        eidx = np.concatenate(
            [_wrap_idx(rids, cfg), _wrap_idx(cids, cfg)], axis=1
        )
        m = {"win": wsh, "eidx": eidx}
        if cfg.z12:
            m["zin8"] = a8
            m["zn4"] = n4
        else:
            m["zin"] = zsh
        per_core.append(m)
    return per_core


_NC_CACHE = {}


def get_nc(cfg: Cfg):
    key = (cfg.gchunk, cfg.big_gather, cfg.gq2, cfg.z12)
    if key not in _NC_CACHE:
        _NC_CACHE[key] = build_kernel(cfg)
    return _NC_CACHE[key]


class CachedRunner:
    """One jitted shard_map executable per module, reused across calls.

    The stock run_bass_kernel_spmd path rebuilds closures and re-traces
    jax.jit on EVERY call (~170ms with this NEFF embedded in the HLO).
    Caching the jitted fn and feeding plain numpy args avoids that; unlike
    the device-resident-input Runner pattern, numpy args do not desync the
    axon mesh (verified over interleaved calls on HW).
    """

    def __init__(self, nc, n_cores: int):
        import jax
        import concourse.mybir as mybir
        from concourse import bass2jax
        from concourse.bass2jax import _bass_exec_p, partition_id_tensor
        from jax.experimental.shard_map import shard_map
        from jax.sharding import Mesh, PartitionSpec

        bass2jax.install_neuronx_cc_hook()
        self.n_cores = n_cores

        in_names, out_names, out_avals, zero_outs = [], [], [], []
        for alloc in nc.m.functions[0].allocations:
            if not isinstance(alloc, mybir.MemoryLocationSet):
                continue
            name = alloc.memorylocations[0].name
            if alloc.kind == "ExternalInput":
                in_names.append(name)
            elif alloc.kind == "ExternalOutput":
                out_names.append(name)
                shape = tuple(alloc.tensor_shape)
                dtype = mybir.dt.np(alloc.dtype)
                out_avals.append(jax.core.ShapedArray(shape, dtype))
                zero_outs.append(
                    np.zeros((n_cores * shape[0], *shape[1:]), dtype)
                )
        partition_name = (
            nc.partition_id_tensor.name if nc.partition_id_tensor else None
        )
        if partition_name is not None:
            in_names.remove(partition_name)
        full_in_names = in_names + out_names
        if partition_name is not None:
            full_in_names.append(partition_name)
        self.in_names = in_names
        self.out_names = out_names
        self.out_avals = out_avals
        self.zero_outs = zero_outs

        def _body(*args):
            operands = list(args)
            if partition_name is not None:
                operands.append(partition_id_tensor())
            outs = _bass_exec_p.bind(
                *operands,
                out_avals=tuple(out_avals),
                in_names=tuple(full_in_names),
                out_names=tuple(out_names),
                lowering_input_output_aliases=(),
                sim_require_finite=True,
                sim_require_nnan=True,
                nc=nc,
            )
            return tuple(outs)

        devices = jax.devices()[:n_cores]
        mesh = Mesh(np.asarray(devices), ("core",))
        n_args = len(in_names) + len(out_names)
        self.fn = jax.jit(
            shard_map(
                _body,
                mesh=mesh,
                in_specs=(PartitionSpec("core"),) * n_args,
                out_specs=(PartitionSpec("core"),) * len(out_names),
                check_rep=False,
            ),
            keep_unused=True,
        )

    def prepare(self, in_maps):
        """Concat per-core inputs once; reuse the result across calls."""
        return [
            np.concatenate(
                [np.asarray(in_maps[c][name]) for c in range(self.n_cores)],
                axis=0,
            )
            for name in self.in_names
        ]

    def call_prepared(self, concat):
        out_arrs = self.fn(*concat, *self.zero_outs)
        return [
            {
                name: np.asarray(out_arrs[i]).reshape(
                    self.n_cores, *self.out_avals[i].shape
                )[c]
                for i, name in enumerate(self.out_names)
            }
            for c in range(self.n_cores)
        ]

    def __call__(self, in_maps):
        return self.call_prepared(self.prepare(in_maps))


_RUNNER_CACHE = {}


def get_runner(cfg: Cfg) -> CachedRunner:
    key = (cfg.gchunk, cfg.big_gather, cfg.gq2, cfg.z12)
    if key not in _RUNNER_CACHE:
        _RUNNER_CACHE[key] = CachedRunner(get_nc(cfg), cfg.n_cores)
    return _RUNNER_CACHE[key]


def _unshard(results, cfg: Cfg) -> np.ndarray:
    parts = []
    for c in range(cfg.n_cores):
        raw = results[c]["scores"]  # [128, eblocks], edge i at [i%128, i//128]
        parts.append(raw.T.reshape(-1)[: cfg.e_core])
    return np.concatenate(parts).astype(np.float32)


def run(z_drug, weight, batch_edges, cfg: Cfg, repeats: int = 1):
    """Returns (scores[200000] f32, [wall seconds per call])."""
    import time

    in_maps = prep_inputs(z_drug, weight, batch_edges, cfg)
    runner = get_runner(cfg)
    concat = runner.prepare(in_maps)
    walls = []
    results = None
    for _ in range(max(1, repeats)):
        t0 = time.perf_counter()
        try:
            results = runner.call_prepared(concat)
        except Exception:
            if results is not None:
                break  # keep earlier good result; a repeat run hiccupped
            # fall back to the slower but battle-tested per-call path
            from concourse.bass_utils import run_bass_kernel_spmd

            time.sleep(30)
            results = run_bass_kernel_spmd(
                get_nc(cfg), in_maps, core_ids=list(range(cfg.n_cores))
            ).results
        walls.append(time.perf_counter() - t0)
    return _unshard(results, cfg), walls


def kernel(z_drug, weight, batch_edges):
    out, _ = run(z_drug, weight, batch_edges, CFG)
    return out
